# revision 9
# baseline (speedup 1.0000x reference)
"""Trainium2 Bass kernel for nn_LBLbiLm: two-stream (left/right context window)
residual MLP stack.

  B,S,H,W,L = 32,1024,512,4,2, fp32.
  padded = [left_pad; x; right_pad]                       # [B, S+2W, H]
  left0[t]  = sum_w lw[w] * padded[t+w]                   # tokens t-4..t-1
  right0[t] = sum_w rw[w] * padded[t+W+1+w]               # tokens t+1..t+4
  per layer l (each stream):  x = x + relu(LN(x) @ w1.T + b1) @ w2.T + b2
  outputs: all_layers [L,S,B,2H] and final [B,S,2H].

Strategy: data-parallel over batch across 8 NeuronCores (4 batches/core).
Token-major residual stream in SBUF; windowed sums + residual adds run on
TensorE via banded shift-matrix / identity matmuls; LayerNorm stats on
VectorE (bn_stats); matmul1 consumes PE-transposed LN output and produces
the hidden feature-major; matmul2 uses the hidden as the stationary operand
so its output lands token-major again.  Matmuls run in float32r (full PE
rate at free dim >= 256, ~11 mantissa bits).  All parameter folding
(w1*gamma, bias folding, shift matrices baked from left/right_weights)
happens host-side in numpy; constants ship as two packed tensors so each
consumer pays at most one DMA-semaphore wait.
"""

import numpy as np

import concourse.bass as bass
import concourse.bacc as bacc
import concourse.tile as tile
import concourse.mybir as mybir
from concourse.bass_utils import run_bass_kernel_spmd

B, S, H, W, L = 32, 1024, 512, 4, 2
NCORES = 8
BL = B // NCORES          # local batches per core
P = 128
TCH = 512                 # tokens per chunk
NT = TCH // P             # t-tiles per chunk
NCH = S // TCH            # chunks per sequence
EPS = 1e-6

F32 = mybir.dt.float32
F32R = mybir.dt.float32r
AF = mybir.ActivationFunctionType
ALU = mybir.AluOpType

# packed const offsets (in f32 elements per partition)
OFF_W = 0                      # [8, 4, 512] weights
OFF_S0 = OFF_W + 8 * 4 * H     # [2, 128]
OFF_S1 = OFF_S0 + 2 * P        # [2, 128]
OFF_ID = OFF_S1 + 2 * P        # [128]
OFF_C1 = OFF_ID + P            # [2, 2, 4] c1 bits (f32)
PK_W = OFF_C1 + 2 * L * 4

# small pack (4 partitions)
SOFF_PADL = 0
SOFF_PADR = SOFF_PADL + H
SOFF_S4L = SOFF_PADR + H
SOFF_S4R = SOFF_S4L + P
SOFF_B2 = SOFF_S4R + P          # row 0 only: [2, 2, H]
SOFF_ONES = SOFF_B2 + 2 * L * H  # row 0 only: [128]


def _sm_width(use_b2):
    return (SOFF_ONES + P) if use_b2 else SOFF_B2


def _build_program(use_b2: bool):
    nc = bacc.Bacc(
        "TRN2",
        target_bir_lowering=False,
        debug=False,
        num_devices=NCORES,
    )

    x_in = nc.dram_tensor("x_in", [BL, S, H], F32R, kind="ExternalInput").ap()
    cpk = nc.dram_tensor("cpk", [P, PK_W], F32R, kind="ExternalInput").ap()
    SM_W = _sm_width(use_b2)
    csm = nc.dram_tensor("csm", [W, SM_W], F32R, kind="ExternalInput").ap()

    out_all = nc.dram_tensor(
        "out_all", [L, S, BL, 2 * H], F32, kind="ExternalOutput"
    ).ap()
    out_last = nc.dram_tensor(
        "out_last", [BL, S, 2 * H], F32, kind="ExternalOutput"
    ).ap()

    with tile.TileContext(nc) as tc:
        with (
            tc.tile_pool(name="const", bufs=1) as const,
            tc.tile_pool(name="xin", bufs=2) as xinp,
            tc.tile_pool(name="halo", bufs=2) as halop,
            tc.tile_pool(name="xres", bufs=6) as xresp,
            tc.tile_pool(name="xn", bufs=2) as xnp,
            tc.tile_pool(name="xnT", bufs=2) as xnTp,
            tc.tile_pool(name="y1", bufs=2) as y1p,
            tc.tile_pool(name="stats", bufs=16) as stats,
            tc.tile_pool(name="pw", bufs=2, space="PSUM") as pwp,
            tc.tile_pool(name="pt", bufs=2, space="PSUM") as ptp,
            tc.tile_pool(name="p1", bufs=2, space="PSUM") as p1p,
            tc.tile_pool(name="p2", bufs=2, space="PSUM") as p2p,
        ):
            # ---- constants: two packed DMAs ----
            pk = const.tile([P, PK_W], F32R)
            nc.sync.dma_start(pk, cpk)
            sm = const.tile([W, SM_W], F32R)
            nc.sync.dma_start(sm, csm)

            wsb = pk[:, OFF_W : OFF_S0].rearrange("p (n c m) -> p n c m", n=8, c=4)
            s0sb = pk[:, OFF_S0 : OFF_S1].rearrange("p (s b) -> p s b", s=2)
            s1sb = pk[:, OFF_S1 : OFF_ID].rearrange("p (s b) -> p s b", s=2)
            idsb = pk[:, OFF_ID : OFF_C1]
            c1sb = pk[:, OFF_C1 : PK_W].rearrange(
                "p (s l c) -> p s l c", s=2, l=L
            ).bitcast(F32)
            padL = sm[:, SOFF_PADL:SOFF_PADR]
            padR = sm[:, SOFF_PADR:SOFF_S4L]
            s4sbL = sm[:, SOFF_S4L:SOFF_S4R]
            s4sbR = sm[:, SOFF_S4R:SOFF_B2]
            if use_b2:
                b2sb = sm[0:1, SOFF_B2:SOFF_ONES].rearrange(
                    "a (s l m) -> a s l m", s=2, l=L
                )
                onesb = sm[0:1, SOFF_ONES : SOFF_ONES + P]

            for b in range(BL):
                for c in range(NCH):
                    xin = xinp.tile([P, NT, H], F32R, tag="xin")
                    nc.sync.dma_start(
                        xin,
                        x_in[b, c * TCH : (c + 1) * TCH, :].rearrange(
                            "(t p) h -> p t h", p=P
                        ),
                    )
                    # 4-row halo tiles (left needs prev rows, right needs next)
                    if c == 0:
                        prevh = padL
                    else:
                        prevh = halop.tile([W, H], F32R, tag="ph")
                        nc.sync.dma_start(prevh, x_in[b, c * TCH - W : c * TCH, :])
                    if c == NCH - 1:
                        nexth = padR
                    else:
                        nexth = halop.tile([W, H], F32R, tag="nh")
                        nc.sync.dma_start(
                            nexth, x_in[b, (c + 1) * TCH : (c + 1) * TCH + W, :]
                        )

                    for side in range(2):
                        # ---- windowed sum -> x0 (token-major, f32r) ----
                        x0 = xresp.tile([P, NT, H], F32R, tag="xres")
                        for tt in range(NT):
                            pw = pwp.tile([P, H], F32, tag="pw")
                            nc.tensor.matmul(
                                pw, s0sb[:, side, :], xin[:, tt, :],
                                start=True, stop=False,
                            )
                            if side == 0:
                                if tt > 0:
                                    nc.tensor.matmul(
                                        pw, s1sb[:, 0, :], xin[:, tt - 1, :],
                                        start=False, stop=True,
                                    )
                                else:
                                    nc.tensor.matmul(
                                        pw, s4sbL, prevh,
                                        start=False, stop=True,
                                    )
                            else:
                                if tt < NT - 1:
                                    nc.tensor.matmul(
                                        pw, s1sb[:, 1, :], xin[:, tt + 1, :],
                                        start=False, stop=True,
                                    )
                                else:
                                    nc.tensor.matmul(
                                        pw, s4sbR, nexth,
                                        start=False, stop=True,
                                    )
                            nc.scalar.activation(x0[:, tt, :], pw, AF.Copy)

                        xcur = x0
                        for l in range(L):
                            # ---- LayerNorm (token-major) ----
                            xn = xnp.tile([P, NT, H], F32, tag="xn")
                            for tt in range(NT):
                                st = stats.tile([P, 6], F32, tag="st")
                                nc.vector.bn_stats(
                                    st, xcur[:, tt, :].bitcast(F32)
                                )
                                mv = stats.tile([P, 2], F32, tag="mv")
                                nc.vector.bn_aggr(mv, st)
                                sd = stats.tile([P, 1], F32, tag="sd")
                                nc.scalar.activation(
                                    sd, mv[:, 1:2], AF.Sqrt,
                                    scale=float(H / (H - 1)),
                                )
                                nc.vector.tensor_scalar(
                                    out=sd, in0=sd, scalar1=EPS, scalar2=None,
                                    op0=ALU.add,
                                )
                                nc.vector.reciprocal(sd, sd)
                                nc.vector.tensor_scalar(
                                    out=xn[:, tt, :],
                                    in0=xcur[:, tt, :].bitcast(F32),
                                    scalar1=mv[:, 0:1],
                                    scalar2=sd,
                                    op0=ALU.subtract,
                                    op1=ALU.mult,
                                )
                            # ---- transpose xn -> xnT [h_p, hc, t] ----
                            xnT = xnTp.tile([P, 4, TCH], F32R, tag="xnT")
                            for hc in range(4):
                                pt = ptp.tile([P, TCH], F32, tag="pt")
                                for tt in range(NT):
                                    nc.tensor.matmul(
                                        pt[:, tt * P : (tt + 1) * P],
                                        xn[:, tt, hc * P : (hc + 1) * P],
                                        idsb.bitcast(F32),
                                        is_transpose=True,
                                        start=(tt == 0),
                                        stop=(tt == NT - 1),
                                    )
                                nc.scalar.activation(xnT[:, hc, :], pt, AF.Copy)

                            # ---- mm1 -> y1 (feature-major, relu+c1 fused) ----
                            j1 = side * 4 + l * 2
                            y1 = y1p.tile([P, 4, TCH], F32R, tag="y1")
                            for mc in range(4):
                                p1 = p1p.tile([P, TCH], F32, tag="p1")
                                for kc in range(4):
                                    nc.tensor.matmul(
                                        p1,
                                        wsb[:, j1, kc, mc * P : (mc + 1) * P],
                                        xnT[:, kc, :],
                                        start=(kc == 0),
                                        stop=(kc == 3),
                                    )
                                nc.scalar.activation(
                                    y1[:, mc, :], p1, AF.Relu,
                                    bias=c1sb[:, side, l, mc : mc + 1],
                                )

                            # ---- mm2 + residual (+b2) -> token-major ----
                            xnext = xresp.tile([P, NT, H], F32R, tag="xres")
                            for tt in range(NT):
                                p2 = p2p.tile([P, H], F32, tag="p2")
                                for kc in range(4):
                                    nc.tensor.matmul(
                                        p2,
                                        y1[:, kc, tt * P : (tt + 1) * P],
                                        wsb[:, j1 + 1, kc, :],
                                        start=(kc == 0),
                                        stop=False,
                                    )
                                nc.tensor.matmul(
                                    p2, idsb, xcur[:, tt, :],
                                    start=False, stop=not use_b2,
                                )
                                if use_b2:
                                    nc.tensor.matmul(
                                        p2, onesb, b2sb[:, side, l, :],
                                        start=False, stop=True,
                                    )
                                nc.scalar.activation(xnext[:, tt, :], p2, AF.Copy)
                            # one DMA per destination for the whole chunk
                            nc.sync.dma_start(
                                out_all[
                                    l, c * TCH : (c + 1) * TCH, b,
                                    side * H : (side + 1) * H,
                                ].rearrange("(t p) h -> p t h", p=P),
                                xnext.bitcast(F32),
                            )
                            if l == L - 1:
                                nc.sync.dma_start(
                                    out_last[
                                        b, c * TCH : (c + 1) * TCH,
                                        side * H : (side + 1) * H,
                                    ].rearrange("(t p) h -> p t h", p=P),
                                    xnext.bitcast(F32),
                                )
                            xcur = xnext
    nc.compile()
    return nc


def _host_prep(inputs):
    """Fold parameters, build shift matrices, pack constants; per-core maps."""
    lw = np.asarray(inputs["left_weights"], np.float32)[:, 0]
    rw = np.asarray(inputs["right_weights"], np.float32)[:, 0]

    w_list = []
    c1_arr = np.zeros((2, L, H), np.float32)
    b2_arr = np.zeros((2, L, H), np.float32)
    for si, pre in enumerate(("l", "r")):
        w1 = np.asarray(inputs[pre + "w1"], np.float32)
        b1 = np.asarray(inputs[pre + "b1"], np.float32)
        w2 = np.asarray(inputs[pre + "w2"], np.float32)
        b2 = np.asarray(inputs[pre + "b2"], np.float32)
        g = np.asarray(inputs[pre + "g"], np.float32)
        bt = np.asarray(inputs[pre + "bt"], np.float32)
        for l in range(L):
            w_list.append(np.ascontiguousarray((w1[l] * g[l][None, :]).T))
            w_list.append(np.ascontiguousarray(w2[l].T))
            c1_arr[si, l] = bt[l] @ w1[l].T + b1[l]
            b2_arr[si, l] = b2[l]
    w_all = np.stack(w_list, 0)  # [8, H, H]: (side, layer, {a1t, a2t})

    sl0 = np.zeros((P, P), np.float32)
    sl1 = np.zeros((P, P), np.float32)
    sr0 = np.zeros((P, P), np.float32)
    sr1 = np.zeros((P, P), np.float32)
    for t in range(P):
        for w in range(W):
            src = t - W + w
            if src >= 0:
                sl0[src, t] += lw[w]
            else:
                sl1[P + src, t] += lw[w]
            src = t + 1 + w
            if src < P:
                sr0[src, t] += rw[w]
            else:
                sr1[src - P, t] += rw[w]

    use_b2 = bool(np.any(b2_arr))

    # ---- pack the [128, ...] constants ----
    pk = np.zeros((P, PK_W), np.float32)
    pk[:, OFF_W:OFF_S0] = (
        w_all.reshape(8, 4, P, H).transpose(2, 0, 1, 3).reshape(P, 8 * 4 * H)
    )
    pk[:, OFF_S0:OFF_S1] = np.stack([sl0, sr0], 1).reshape(P, 2 * P)
    pk[:, OFF_S1:OFF_ID] = np.stack([sl1, sr1], 1).reshape(P, 2 * P)
    pk[:, OFF_ID:OFF_C1] = np.eye(P, dtype=np.float32)
    pk[:, OFF_C1:PK_W] = (
        c1_arr.reshape(2, L, 4, P).transpose(3, 0, 1, 2).reshape(P, 2 * L * 4)
    )

    # ---- pack the small (4-partition) constants ----
    sm = np.zeros((W, _sm_width(use_b2)), np.float32)
    sm[:, SOFF_PADL:SOFF_PADR] = np.asarray(inputs["left_padding"], np.float32)
    sm[:, SOFF_PADR:SOFF_S4L] = np.asarray(inputs["right_padding"], np.float32)
    sm[:, SOFF_S4L:SOFF_S4R] = sl1[P - W : P, :]
    sm[:, SOFF_S4R:SOFF_B2] = sr1[0:W, :]
    if use_b2:
        sm[0, SOFF_B2:SOFF_ONES] = b2_arr.reshape(-1)
        sm[0, SOFF_ONES : SOFF_ONES + P] = 1.0

    x = np.asarray(inputs["inputs"], np.float32)
    shared = {"cpk": np.ascontiguousarray(pk), "csm": np.ascontiguousarray(sm)}
    in_maps = []
    for core in range(NCORES):
        m = dict(shared)
        m["x_in"] = np.ascontiguousarray(x[core * BL : (core + 1) * BL])
        in_maps.append(m)
    return in_maps, use_b2


_PROGRAM_CACHE = {}


def _get_program(use_b2: bool):
    if use_b2 not in _PROGRAM_CACHE:
        _PROGRAM_CACHE[use_b2] = _build_program(use_b2)
    return _PROGRAM_CACHE[use_b2]


def kernel(**inputs):
    in_maps, use_b2 = _host_prep(inputs)
    nc = _get_program(use_b2)
    res = run_bass_kernel_spmd(nc, in_maps, core_ids=list(range(NCORES)))
    all_layers = np.concatenate([r["out_all"] for r in res.results], axis=2)
    last = np.concatenate([r["out_last"] for r in res.results], axis=0)
    return all_layers, last
